# revision 3
# baseline (speedup 1.0000x reference)
"""MoE (ExpertPool) expert-parallel kernel for Trainium2, 8 NeuronCores.

Strategy (per sharding hint): expert-parallel. Host computes the (tiny)
router: logits = x@Wr+br, top-2 selection, softmax combine weights. Tokens
are gathered per expert on the host ("dispatch"), each of the 8 experts'
token batch + weights go to one NeuronCore, which runs the 3-layer GELU MLP
and scales rows by the combine weight. Host scatter-adds the per-expert
outputs back ("combine").

Device kernel layout: everything feature-major (features on SBUF
partitions, tokens on the free dim). For each token chunk (<=768 tokens),
the full W1/W2/W3 stream through SBUF as 128-column panels while the PE
does float32r matmuls (full-rate fp32). GELU+bias fused into ScalarE
activation ops reading PSUM. Gating is a DVE multiply against a
partition-broadcast gate row. Output is written feature-major [D, C] and
transposed on the host during the combine.
"""

import numpy as np

# Problem dims (hardcoded per spec: nn_ExpertPool_8366596292698)
B, S, D, E, I = 8, 2048, 768, 8, 3072
H = I // 2
T = B * S
P = 128
KD, KI, KH = D // P, I // P, H // P  # 6, 24, 12
N_CORES = 8
DEFAULT_CAP = 4224  # observed max expert load for the fixed harness inputs

_PROGRAM_CACHE: dict = {}
LAST_RESULTS = None  # BassKernelResults of the most recent run (for test harness)


def _chunk_sizes(C):
    """Split C (multiple of 128, >=256) into chunks <=768, each chunk
    decomposable into matmul column groups of >=256 columns."""
    sizes = []
    rem = C
    while rem:
        if rem == 896:
            take = 640
        elif rem >= 768:
            take = 768
        else:
            take = rem  # 256/384/512/640
        sizes.append(take)
        rem -= take
    assert all(256 <= s <= 768 and s % 128 == 0 for s in sizes), sizes
    return sizes


def _col_groups(nc_tokens):
    """Column groups (start, len) of a chunk; each >=256 (f32r full rate)."""
    if nc_tokens <= 512:
        return [(0, nc_tokens)]
    if nc_tokens == 640:
        return [(0, 384), (384, 256)]
    return [(0, 512), (512, 256)]


def _build_program(C, has_b1, has_b2, has_b3):
    from contextlib import ExitStack

    import concourse.bacc as bacc
    import concourse.bass as bass
    import concourse.mybir as mybir
    import concourse.tile as tile

    f32 = mybir.dt.float32
    f32r = mybir.dt.float32r
    GELU = mybir.ActivationFunctionType.Gelu

    nc = bacc.Bacc(
        "TRN2",
        target_bir_lowering=False,
        debug=False,
        enable_asserts=False,
        num_devices=N_CORES,
    )

    xT = nc.dram_tensor("xT", [D, C], f32r, kind="ExternalInput").ap()
    w1 = nc.dram_tensor("w1", [D, I], f32r, kind="ExternalInput").ap()
    w2 = nc.dram_tensor("w2", [I, H], f32r, kind="ExternalInput").ap()
    w3 = nc.dram_tensor("w3", [H, D], f32r, kind="ExternalInput").ap()
    gate = nc.dram_tensor("gate", [C], f32, kind="ExternalInput").ap()
    b1 = b2 = b3 = None
    if has_b1:
        b1 = nc.dram_tensor("b1t", [P, KI], f32, kind="ExternalInput").ap()
    if has_b2:
        b2 = nc.dram_tensor("b2t", [P, KH], f32, kind="ExternalInput").ap()
    if has_b3:
        b3 = nc.dram_tensor("b3t", [P, KD], f32, kind="ExternalInput").ap()
    yT = nc.dram_tensor("yT", [D, C], f32, kind="ExternalOutput").ap()

    chunks = _chunk_sizes(C)

    with tile.TileContext(nc) as tc, ExitStack() as ctx:
        const_pool = ctx.enter_context(tc.tile_pool(name="const", bufs=1))
        xpool = ctx.enter_context(tc.tile_pool(name="x", bufs=1))
        h1pool = ctx.enter_context(tc.tile_pool(name="h1", bufs=1))
        h2pool = ctx.enter_context(tc.tile_pool(name="h2", bufs=1))
        wpool = ctx.enter_context(tc.tile_pool(name="w", bufs=2))
        ypool = ctx.enter_context(tc.tile_pool(name="y", bufs=2))
        gpool = ctx.enter_context(tc.tile_pool(name="g", bufs=2))
        pspool = ctx.enter_context(
            tc.tile_pool(name="ps", bufs=4, space=bass.MemorySpace.PSUM)
        )

        b1_sb = b2_sb = b3_sb = None
        if has_b1:
            b1_sb = const_pool.tile([P, KI], f32, tag="b1")
            nc.scalar.dma_start(b1_sb[:], b1[:, :])
        if has_b2:
            b2_sb = const_pool.tile([P, KH], f32, tag="b2")
            nc.scalar.dma_start(b2_sb[:], b2[:, :])
        if has_b3:
            b3_sb = const_pool.tile([P, KD], f32, tag="b3")
            nc.scalar.dma_start(b3_sb[:], b3[:, :])

        base = 0
        for Nc in chunks:
            cgs = _col_groups(Nc)

            # token activations for this chunk, feature-major [128, Nc] x 6
            x_sb = []
            for k in range(KD):
                xk = xpool.tile([P, Nc], f32r, tag=f"x{k}")
                nc.scalar.dma_start(xk[:], xT[k * P : (k + 1) * P, base : base + Nc])
                x_sb.append(xk)

            # gate row broadcast to all 128 partitions, [128, Nc]
            g_bc = gpool.tile([P, Nc], f32, tag="gbc")
            nc.scalar.dma_start(
                g_bc[:],
                gate[base : base + Nc].unsqueeze(0).partition_broadcast(P).squeeze(1),
            )

            # ---- L1: h1 = gelu(x @ W1 + b1), feature-major [I, Nc] ----
            h1_sb = []
            for m in range(KI):
                w1p = wpool.tile([P, KD * P], f32r, tag="w1p")
                nc.sync.dma_start(
                    w1p[:].rearrange("p (a f) -> p a f", f=P),
                    w1[:, m * P : (m + 1) * P].rearrange("(a p) f -> p a f", p=P),
                )
                ps = pspool.tile([P, Nc], f32, tag="ps")
                for cs, cn in cgs:
                    for k in range(KD):
                        nc.tensor.matmul(
                            ps[:, cs : cs + cn],
                            lhsT=w1p[:, k * P : (k + 1) * P],
                            rhs=x_sb[k][:, cs : cs + cn],
                            start=(k == 0),
                            stop=(k == KD - 1),
                        )
                h1m = h1pool.tile([P, Nc], f32r, tag=f"h1_{m}")
                nc.scalar.activation(
                    h1m[:],
                    ps[:],
                    GELU,
                    bias=(b1_sb[:, m : m + 1] if has_b1 else 0.0),
                )
                h1_sb.append(h1m)

            # ---- L2: h2 = gelu(h1 @ W2 + b2), feature-major [H, Nc] ----
            h2_sb = []
            for m in range(KH):
                w2p = wpool.tile([P, KI * P], f32r, tag="w2p")
                nc.sync.dma_start(
                    w2p[:].rearrange("p (a f) -> p a f", f=P),
                    w2[:, m * P : (m + 1) * P].rearrange("(a p) f -> p a f", p=P),
                )
                ps = pspool.tile([P, Nc], f32, tag="ps")
                for cs, cn in cgs:
                    for k in range(KI):
                        nc.tensor.matmul(
                            ps[:, cs : cs + cn],
                            lhsT=w2p[:, k * P : (k + 1) * P],
                            rhs=h1_sb[k][:, cs : cs + cn],
                            start=(k == 0),
                            stop=(k == KI - 1),
                        )
                h2m = h2pool.tile([P, Nc], f32r, tag=f"h2_{m}")
                nc.scalar.activation(
                    h2m[:],
                    ps[:],
                    GELU,
                    bias=(b2_sb[:, m : m + 1] if has_b2 else 0.0),
                )
                h2_sb.append(h2m)

            # ---- L3: y = (h2 @ W3 + b3) * gate, feature-major [D, Nc] ----
            for m in range(KD):
                w3p = wpool.tile([P, KH * P], f32r, tag="w3p")
                nc.sync.dma_start(
                    w3p[:].rearrange("p (a f) -> p a f", f=P),
                    w3[:, m * P : (m + 1) * P].rearrange("(a p) f -> p a f", p=P),
                )
                ps = pspool.tile([P, Nc], f32, tag="ps")
                for cs, cn in cgs:
                    for k in range(KH):
                        nc.tensor.matmul(
                            ps[:, cs : cs + cn],
                            lhsT=w3p[:, k * P : (k + 1) * P],
                            rhs=h2_sb[k][:, cs : cs + cn],
                            start=(k == 0),
                            stop=(k == KH - 1),
                        )
                y_sb = ypool.tile([P, Nc], f32, tag="y")
                if has_b3:
                    nc.vector.tensor_scalar_add(y_sb[:], ps[:], b3_sb[:, m : m + 1])
                    nc.vector.tensor_mul(y_sb[:], y_sb[:], g_bc[:])
                else:
                    nc.vector.tensor_mul(y_sb[:], ps[:], g_bc[:])
                nc.scalar.dma_start(yT[m * P : (m + 1) * P, base : base + Nc], y_sb[:])

            base += Nc

    nc.compile()
    return nc


def _route(x, Wr, br, top_k):
    """Host router: fp32 logits, stable top-k, softmax weights."""
    xt = np.ascontiguousarray(x.reshape(T, D), dtype=np.float32)
    logits = (xt @ np.asarray(Wr, np.float32)) + np.asarray(br, np.float32)
    k = int(top_k)
    # descending by value, ties -> lower index (matches jax.lax.top_k)
    order = np.argsort(-logits, axis=1, kind="stable")[:, :k]  # [T, k]
    vals = np.take_along_axis(logits, order, axis=1)
    vmax = vals.max(axis=1, keepdims=True)
    ex = np.exp(vals - vmax)
    wts = (ex / ex.sum(axis=1, keepdims=True)).astype(np.float32)
    return xt, order, wts


def kernel(x, Wr, br, W1, b1, W2, b2, W3, b3, top_k):
    global LAST_RESULTS
    import os

    from concourse import bass_utils

    x = np.asarray(x)
    out_dtype = x.dtype
    xt, sel, wts = _route(x, Wr, br, top_k)

    W1 = np.asarray(W1, np.float32)
    W2 = np.asarray(W2, np.float32)
    W3 = np.asarray(W3, np.float32)
    b1 = np.asarray(b1, np.float32)
    b2 = np.asarray(b2, np.float32)
    b3 = np.asarray(b3, np.float32)

    # token lists per expert
    idx_e = []
    gate_e = []
    for e in range(E):
        rows, cols = np.nonzero(sel == e)
        idx_e.append(rows)
        gate_e.append(wts[rows, cols])
    counts = np.array([len(i) for i in idx_e])

    C = max(DEFAULT_CAP, int(-(-counts.max() // P) * P), 256)

    has_b1 = bool(np.any(b1))
    has_b2 = bool(np.any(b2))
    has_b3 = bool(np.any(b3))

    key = (C, has_b1, has_b2, has_b3)
    if key not in _PROGRAM_CACHE:
        _PROGRAM_CACHE[key] = _build_program(C, has_b1, has_b2, has_b3)
    nc = _PROGRAM_CACHE[key]

    # biggest expert goes to core 0 (the traced core)
    perm = np.argsort(-counts, kind="stable")

    in_maps = []
    for i in range(N_CORES):
        e = int(perm[i])
        n = int(counts[e])
        xTe = np.zeros((D, C), np.float32)
        xTe[:, :n] = xt[idx_e[e]].T
        g = np.zeros((C,), np.float32)
        g[:n] = gate_e[e]
        m = {
            "xT": xTe,
            "w1": W1[e],
            "w2": W2[e],
            "w3": W3[e],
            "gate": g,
        }
        if has_b1:
            m["b1t"] = np.ascontiguousarray(b1[e].reshape(KI, P).T)
        if has_b2:
            m["b2t"] = np.ascontiguousarray(b2[e].reshape(KH, P).T)
        if has_b3:
            m["b3t"] = np.ascontiguousarray(b3[e].reshape(KD, P).T)
        in_maps.append(m)

    trace_cores = None
    if os.environ.get("BASS_TRACE"):
        trace_cores = [0]

    res = bass_utils.run_bass_kernel_spmd(
        nc,
        in_maps,
        core_ids=list(range(N_CORES)),
        trace_cores=trace_cores,
    )
    LAST_RESULTS = res

    out = np.zeros((T, D), np.float32)
    core_of_expert = {int(perm[i]): i for i in range(N_CORES)}
    for e in range(E):  # expert-ascending to match reference summation order
        i = core_of_expert[e]
        n = int(counts[e])
        if n:
            out[idx_e[e]] += res.results[i]["yT"][:, :n].T
    return out.reshape(B, S, D).astype(out_dtype, copy=False)


# revision 10
# speedup vs baseline: 1.0001x; 1.0001x over previous
"""MoE (ExpertPool) expert-parallel kernel for Trainium2, 8 NeuronCores.

Strategy (per sharding hint): expert-parallel. Host computes the (tiny)
router: logits = x@Wr+br, top-2 selection, softmax combine weights. Tokens
are gathered per expert on the host ("dispatch"), each of the 8 experts'
token batch + weights go to one NeuronCore, which runs the 3-layer GELU MLP
and scales rows by the combine weight. Host scatter-adds the per-expert
outputs back ("combine").

Device kernel layout: everything feature-major (features on SBUF
partitions, tokens on the free dim). For each token chunk (<=768 tokens),
the full W1/W2/W3 stream through SBUF as 128-column panels while the PE
does float32r matmuls (full-rate fp32). GELU+bias fused into ScalarE
activation ops reading PSUM. Gating is a DVE multiply against a
partition-broadcast gate row. Output is written feature-major [D, C] and
transposed on the host during the combine.
"""

import numpy as np

# Problem dims (hardcoded per spec: nn_ExpertPool_8366596292698)
B, S, D, E, I = 8, 2048, 768, 8, 3072
H = I // 2
T = B * S
P = 128
KD, KI, KH = D // P, I // P, H // P  # 6, 24, 12
N_CORES = 8
DEFAULT_CAP = 4224  # observed max expert load for the fixed harness inputs

_PROGRAM_CACHE: dict = {}
LAST_RESULTS = None  # BassKernelResults of the most recent run (for test harness)


def _chunk_sizes(C):
    """Split C (multiple of 128, >=256) into chunks from {256,384,512,768}.

    Sizes are chosen so every matmul column group is >=256 wide (f32r full
    rate) AND starts bank-aligned in PSUM (512-float banks): 768=(512,256),
    512/384/256 single-group. 640 is forbidden — its (384,256) split would
    cross a PSUM bank boundary."""
    sizes = []
    rem = C
    while rem:
        if rem >= 1280 or rem == 768:
            take = 768
        elif rem in (1152,):
            take = 768
        elif rem == 1024 or rem == 896:
            take = 512
        elif rem == 640:
            take = 384
        else:
            take = rem  # 512/384/256
        sizes.append(take)
        rem -= take
    assert all(s in (256, 384, 512, 768) for s in sizes), (C, sizes)
    assert sum(sizes) == C
    return sizes


def _col_groups(nc_tokens):
    """Column groups (start, len): >=256 wide, PSUM-bank-aligned starts."""
    if nc_tokens <= 512:
        return [(0, nc_tokens)]
    assert nc_tokens == 768
    return [(0, 512), (512, 256)]


def _build_program(C, has_b1, has_b2, has_b3):
    from contextlib import ExitStack

    import concourse.bacc as bacc
    import concourse.bass as bass
    import concourse.mybir as mybir
    import concourse.tile as tile

    f32 = mybir.dt.float32
    f32r = mybir.dt.float32r
    GELU = mybir.ActivationFunctionType.Gelu

    nc = bacc.Bacc(
        "TRN2",
        target_bir_lowering=False,
        debug=False,
        enable_asserts=False,
        num_devices=N_CORES,
    )

    xT = nc.dram_tensor("xT", [D, C], f32r, kind="ExternalInput").ap()
    w1 = nc.dram_tensor("w1", [D, I], f32r, kind="ExternalInput").ap()
    w2 = nc.dram_tensor("w2", [I, H], f32r, kind="ExternalInput").ap()
    w3 = nc.dram_tensor("w3", [H, D], f32r, kind="ExternalInput").ap()
    gate = nc.dram_tensor("gate", [C], f32, kind="ExternalInput").ap()
    b1 = b2 = b3 = None
    if has_b1:
        b1 = nc.dram_tensor("b1t", [P, KI], f32, kind="ExternalInput").ap()
    if has_b2:
        b2 = nc.dram_tensor("b2t", [P, KH], f32, kind="ExternalInput").ap()
    if has_b3:
        b3 = nc.dram_tensor("b3t", [P, KD], f32, kind="ExternalInput").ap()
    yT = nc.dram_tensor("yT", [D, C], f32, kind="ExternalOutput").ap()

    chunks = _chunk_sizes(C)

    with tile.TileContext(nc) as tc, ExitStack() as ctx:
        const_pool = ctx.enter_context(tc.tile_pool(name="const", bufs=1))
        xpool = ctx.enter_context(tc.tile_pool(name="x", bufs=1))
        h1pool = ctx.enter_context(tc.tile_pool(name="h1", bufs=1))
        h2pool = ctx.enter_context(tc.tile_pool(name="h2", bufs=1))
        w1pool = ctx.enter_context(tc.tile_pool(name="w1", bufs=2))
        w2pool = ctx.enter_context(tc.tile_pool(name="w2", bufs=3))
        w3pool = ctx.enter_context(tc.tile_pool(name="w3", bufs=2))
        ypool = ctx.enter_context(tc.tile_pool(name="y", bufs=1))
        gpool = ctx.enter_context(tc.tile_pool(name="g", bufs=2))
        pspool = ctx.enter_context(
            tc.tile_pool(name="ps", bufs=4, space=bass.MemorySpace.PSUM)
        )

        b1_sb = b2_sb = b3_sb = None
        if has_b1:
            b1_sb = const_pool.tile([P, KI], f32, tag="b1")
            nc.scalar.dma_start(b1_sb[:], b1[:, :])
        if has_b2:
            b2_sb = const_pool.tile([P, KH], f32, tag="b2")
            nc.scalar.dma_start(b2_sb[:], b2[:, :])
        if has_b3:
            b3_sb = const_pool.tile([P, KD], f32, tag="b3")
            nc.scalar.dma_start(b3_sb[:], b3[:, :])

        # chunk start offsets
        bases = []
        b_ = 0
        for s in chunks:
            bases.append(b_)
            b_ += s

        def load_chunk_inputs(ci):
            """DMA this chunk's token activations + gate row (ACT HWDGE)."""
            Nc = chunks[ci]
            base = bases[ci]
            x_sb = []
            for k in range(KD):
                xk = xpool.tile([P, Nc], f32r, tag=f"x{k}")
                nc.scalar.dma_start(xk[:], xT[k * P : (k + 1) * P, base : base + Nc])
                x_sb.append(xk)
            g_bc = gpool.tile([P, Nc], f32, tag="gbc")
            nc.scalar.dma_start(
                g_bc[:],
                gate[base : base + Nc].unsqueeze(0).partition_broadcast(P).squeeze(1),
            )
            return x_sb, g_bc

        pending = load_chunk_inputs(0)
        for ci, Nc in enumerate(chunks):
            base = bases[ci]
            cgs = _col_groups(Nc)
            x_sb, g_bc = pending

            # ---- L1: h1 = gelu(x @ W1 + b1), feature-major [I, Nc] ----
            h1_sb = []
            for m in range(KI):
                w1p = w1pool.tile([P, KD * P], f32r, tag="w1p")
                nc.sync.dma_start(
                    w1p[:].rearrange("p (a f) -> p a f", f=P),
                    w1[:, m * P : (m + 1) * P].rearrange("(a p) f -> p a f", p=P),
                )
                ps = pspool.tile([P, Nc], f32, tag="ps")
                for cs, cn in cgs:
                    for k in range(KD):
                        nc.tensor.matmul(
                            ps[:, cs : cs + cn],
                            lhsT=w1p[:, k * P : (k + 1) * P],
                            rhs=x_sb[k][:, cs : cs + cn],
                            start=(k == 0),
                            stop=(k == KD - 1),
                        )
                h1m = h1pool.tile([P, Nc], f32r, tag=f"h1_{m}")
                nc.scalar.activation(
                    h1m[:],
                    ps[:],
                    GELU,
                    bias=(b1_sb[:, m : m + 1] if has_b1 else 0.0),
                )
                h1_sb.append(h1m)

            # prefetch next chunk's activations; their x slots free as soon
            # as this chunk's L1 matmuls finish, so the DMA lands during L2
            if ci + 1 < len(chunks):
                pending = load_chunk_inputs(ci + 1)

            # ---- L2: h2 = gelu(h1 @ W2 + b2), feature-major [H, Nc] ----
            h2_sb = []
            for m in range(KH):
                w2p = w2pool.tile([P, KI * P], f32r, tag="w2p")
                nc.sync.dma_start(
                    w2p[:].rearrange("p (a f) -> p a f", f=P),
                    w2[:, m * P : (m + 1) * P].rearrange("(a p) f -> p a f", p=P),
                )
                ps = pspool.tile([P, Nc], f32, tag="ps")
                for cs, cn in cgs:
                    for k in range(KI):
                        nc.tensor.matmul(
                            ps[:, cs : cs + cn],
                            lhsT=w2p[:, k * P : (k + 1) * P],
                            rhs=h1_sb[k][:, cs : cs + cn],
                            start=(k == 0),
                            stop=(k == KI - 1),
                        )
                h2m = h2pool.tile([P, Nc], f32r, tag=f"h2_{m}")
                nc.scalar.activation(
                    h2m[:],
                    ps[:],
                    GELU,
                    bias=(b2_sb[:, m : m + 1] if has_b2 else 0.0),
                )
                h2_sb.append(h2m)

            # ---- L3: y = (h2 @ W3 + b3) * gate, feature-major [D, Nc] ----
            for m in range(KD):
                w3p = w3pool.tile([P, KH * P], f32r, tag="w3p")
                nc.sync.dma_start(
                    w3p[:].rearrange("p (a f) -> p a f", f=P),
                    w3[:, m * P : (m + 1) * P].rearrange("(a p) f -> p a f", p=P),
                )
                ps = pspool.tile([P, Nc], f32, tag="ps")
                for cs, cn in cgs:
                    for k in range(KH):
                        nc.tensor.matmul(
                            ps[:, cs : cs + cn],
                            lhsT=w3p[:, k * P : (k + 1) * P],
                            rhs=h2_sb[k][:, cs : cs + cn],
                            start=(k == 0),
                            stop=(k == KH - 1),
                        )
                y_sb = ypool.tile([P, Nc], f32, tag="y")
                if has_b3:
                    nc.vector.tensor_scalar_add(y_sb[:], ps[:], b3_sb[:, m : m + 1])
                    nc.vector.tensor_mul(y_sb[:], y_sb[:], g_bc[:])
                else:
                    nc.vector.tensor_mul(y_sb[:], ps[:], g_bc[:])
                nc.scalar.dma_start(yT[m * P : (m + 1) * P, base : base + Nc], y_sb[:])

    nc.compile()
    return nc


def _route(x, Wr, br, top_k):
    """Host router: fp32 logits, stable top-k, softmax weights."""
    xt = np.ascontiguousarray(x.reshape(T, D), dtype=np.float32)
    logits = (xt @ np.asarray(Wr, np.float32)) + np.asarray(br, np.float32)
    k = int(top_k)
    # descending by value, ties -> lower index (matches jax.lax.top_k)
    order = np.argsort(-logits, axis=1, kind="stable")[:, :k]  # [T, k]
    vals = np.take_along_axis(logits, order, axis=1)
    vmax = vals.max(axis=1, keepdims=True)
    ex = np.exp(vals - vmax)
    wts = (ex / ex.sum(axis=1, keepdims=True)).astype(np.float32)
    return xt, order, wts


def kernel(x, Wr, br, W1, b1, W2, b2, W3, b3, top_k):
    global LAST_RESULTS
    import os

    from concourse import bass_utils

    x = np.asarray(x)
    out_dtype = x.dtype
    xt, sel, wts = _route(x, Wr, br, top_k)

    W1 = np.asarray(W1, np.float32)
    W2 = np.asarray(W2, np.float32)
    W3 = np.asarray(W3, np.float32)
    b1 = np.asarray(b1, np.float32)
    b2 = np.asarray(b2, np.float32)
    b3 = np.asarray(b3, np.float32)

    # token lists per expert
    idx_e = []
    gate_e = []
    for e in range(E):
        rows, cols = np.nonzero(sel == e)
        idx_e.append(rows)
        gate_e.append(wts[rows, cols])
    counts = np.array([len(i) for i in idx_e])

    C = max(DEFAULT_CAP, int(-(-counts.max() // P) * P), 256)

    has_b1 = bool(np.any(b1))
    has_b2 = bool(np.any(b2))
    has_b3 = bool(np.any(b3))

    key = (C, has_b1, has_b2, has_b3)
    if key not in _PROGRAM_CACHE:
        _PROGRAM_CACHE[key] = _build_program(C, has_b1, has_b2, has_b3)
    nc = _PROGRAM_CACHE[key]

    # biggest expert goes to core 0 (the traced core)
    perm = np.argsort(-counts, kind="stable")

    in_maps = []
    for i in range(N_CORES):
        e = int(perm[i])
        n = int(counts[e])
        xTe = np.zeros((D, C), np.float32)
        xTe[:, :n] = xt[idx_e[e]].T
        g = np.zeros((C,), np.float32)
        g[:n] = gate_e[e]
        m = {
            "xT": xTe,
            "w1": W1[e],
            "w2": W2[e],
            "w3": W3[e],
            "gate": g,
        }
        if has_b1:
            m["b1t"] = np.ascontiguousarray(b1[e].reshape(KI, P).T)
        if has_b2:
            m["b2t"] = np.ascontiguousarray(b2[e].reshape(KH, P).T)
        if has_b3:
            m["b3t"] = np.ascontiguousarray(b3[e].reshape(KD, P).T)
        in_maps.append(m)

    trace_cores = None
    if os.environ.get("BASS_TRACE"):
        trace_cores = [0]

    res = bass_utils.run_bass_kernel_spmd(
        nc,
        in_maps,
        core_ids=list(range(N_CORES)),
        trace_cores=trace_cores,
    )
    LAST_RESULTS = res

    out = np.zeros((T, D), np.float32)
    core_of_expert = {int(perm[i]): i for i in range(N_CORES)}
    for e in range(E):  # expert-ascending to match reference summation order
        i = core_of_expert[e]
        n = int(counts[e])
        if n:
            out[idx_e[e]] += res.results[i]["yT"][:, :n].T
    return out.reshape(B, S, D).astype(out_dtype, copy=False)


# revision 12
# speedup vs baseline: 1.1486x; 1.1484x over previous
"""MoE (ExpertPool) expert-parallel kernel for Trainium2, 8 NeuronCores.

Strategy (per sharding hint): expert-parallel. Host computes the (tiny)
router: logits = x@Wr+br, top-2 selection, softmax combine weights. Tokens
are gathered per expert on the host ("dispatch"), each of the 8 experts'
token batch + weights go to one NeuronCore, which runs the 3-layer GELU MLP
and scales rows by the combine weight. Host scatter-adds the per-expert
outputs back ("combine").

Device kernel layout: everything feature-major (features on SBUF
partitions, tokens on the free dim). For each token chunk (<=768 tokens),
the full W1/W2/W3 stream through SBUF as 128-column panels while the PE
does float32r matmuls (full-rate fp32). GELU+bias fused into ScalarE
activation ops reading PSUM. Gating is a DVE multiply against a
partition-broadcast gate row. Output is written feature-major [D, C] and
transposed on the host during the combine.
"""

import numpy as np

# Problem dims (hardcoded per spec: nn_ExpertPool_8366596292698)
B, S, D, E, I = 8, 2048, 768, 8, 3072
H = I // 2
T = B * S
P = 128
KD, KI, KH = D // P, I // P, H // P  # 6, 24, 12
N_CORES = 8
DEFAULT_CAP = 4224  # observed max expert load for the fixed harness inputs

_PROGRAM_CACHE: dict = {}
LAST_RESULTS = None  # BassKernelResults of the most recent run (for test harness)


def _chunk_sizes(C):
    """Split C (multiple of 128, >=256) into chunks from {256,384,512,768}.

    Sizes are chosen so every matmul column group is >=256 wide (f32r full
    rate) AND starts bank-aligned in PSUM (512-float banks): 768=(512,256),
    512/384/256 single-group. 640 is forbidden — its (384,256) split would
    cross a PSUM bank boundary."""
    sizes = []
    rem = C
    while rem:
        if rem >= 1280 or rem == 768:
            take = 768
        elif rem in (1152,):
            take = 768
        elif rem == 1024 or rem == 896:
            take = 512
        elif rem == 640:
            take = 384
        else:
            take = rem  # 512/384/256
        sizes.append(take)
        rem -= take
    assert all(s in (256, 384, 512, 768) for s in sizes), (C, sizes)
    assert sum(sizes) == C
    return sizes


def _col_groups(nc_tokens):
    """Column groups (start, len): >=256 wide, PSUM-bank-aligned starts."""
    if nc_tokens <= 512:
        return [(0, nc_tokens)]
    assert nc_tokens == 768
    return [(0, 512), (512, 256)]


def _build_program(C, has_b1, has_b2, has_b3):
    from contextlib import ExitStack

    import concourse.bacc as bacc
    import concourse.bass as bass
    import concourse.mybir as mybir
    import concourse.tile as tile

    f32 = mybir.dt.float32
    f32r = mybir.dt.float32r
    GELU = mybir.ActivationFunctionType.Gelu

    nc = bacc.Bacc(
        "TRN2",
        target_bir_lowering=False,
        debug=False,
        enable_asserts=False,
        num_devices=N_CORES,
    )

    # host-pretiled layouts: every DMA below reads/writes one fully
    # contiguous block (strided 512B-row reads ran the SDMA engines at
    # ~36% of line rate)
    xT = nc.dram_tensor("xTt", [D * C], f32r, kind="ExternalInput").ap()
    w1 = nc.dram_tensor("w1t", [KI, P, KD * P], f32r, kind="ExternalInput").ap()
    w2 = nc.dram_tensor("w2t", [KH, P, KI * P], f32r, kind="ExternalInput").ap()
    w3 = nc.dram_tensor("w3t", [KD, P, KH * P], f32r, kind="ExternalInput").ap()
    gate = nc.dram_tensor("gate", [C], f32, kind="ExternalInput").ap()
    b1 = b2 = b3 = None
    if has_b1:
        b1 = nc.dram_tensor("b1t", [P, KI], f32, kind="ExternalInput").ap()
    if has_b2:
        b2 = nc.dram_tensor("b2t", [P, KH], f32, kind="ExternalInput").ap()
    if has_b3:
        b3 = nc.dram_tensor("b3t", [P, KD], f32, kind="ExternalInput").ap()
    yT = nc.dram_tensor("yTt", [D * C], f32, kind="ExternalOutput").ap()

    chunks = _chunk_sizes(C)

    with tile.TileContext(nc) as tc, ExitStack() as ctx:
        const_pool = ctx.enter_context(tc.tile_pool(name="const", bufs=1))
        xpool = ctx.enter_context(tc.tile_pool(name="x", bufs=1))
        h1pool = ctx.enter_context(tc.tile_pool(name="h1", bufs=1))
        h2pool = ctx.enter_context(tc.tile_pool(name="h2", bufs=1))
        w1pool = ctx.enter_context(tc.tile_pool(name="w1p", bufs=3))
        w2pool = ctx.enter_context(tc.tile_pool(name="w2p", bufs=2))
        w3pool = ctx.enter_context(tc.tile_pool(name="w3p", bufs=3))
        ypool = ctx.enter_context(tc.tile_pool(name="y", bufs=1))
        gpool = ctx.enter_context(tc.tile_pool(name="g", bufs=2))
        pspool = ctx.enter_context(
            tc.tile_pool(name="ps", bufs=8, space=bass.MemorySpace.PSUM)
        )

        b1_sb = b2_sb = b3_sb = None
        if has_b1:
            b1_sb = const_pool.tile([P, KI], f32, tag="b1")
            nc.scalar.dma_start(b1_sb[:], b1[:, :])
        if has_b2:
            b2_sb = const_pool.tile([P, KH], f32, tag="b2")
            nc.scalar.dma_start(b2_sb[:], b2[:, :])
        if has_b3:
            b3_sb = const_pool.tile([P, KD], f32, tag="b3")
            nc.scalar.dma_start(b3_sb[:], b3[:, :])

        # chunk start offsets
        bases = []
        b_ = 0
        for s in chunks:
            bases.append(b_)
            b_ += s

        def load_chunk_inputs(ci):
            """DMA this chunk's token activations + gate row (ACT HWDGE)."""
            Nc = chunks[ci]
            base = bases[ci]
            x_sb = []
            for k in range(KD):
                xk = xpool.tile([P, Nc], f32r, tag=f"x{k}")
                off = (base * D) + k * P * Nc
                nc.scalar.dma_start(
                    xk[:], xT[off : off + P * Nc].rearrange("(p f) -> p f", f=Nc)
                )
                x_sb.append(xk)
            g_bc = gpool.tile([P, Nc], f32, tag="gbc")
            nc.scalar.dma_start(
                g_bc[:],
                gate[base : base + Nc].unsqueeze(0).partition_broadcast(P).squeeze(1),
            )
            return x_sb, g_bc

        pending = load_chunk_inputs(0)
        for ci, Nc in enumerate(chunks):
            base = bases[ci]
            cgs = _col_groups(Nc)
            x_sb, g_bc = pending

            # ---- L1: h1 = gelu(x @ W1 + b1), feature-major [I, Nc] ----
            h1_sb = []
            for m in range(KI):
                w1p = w1pool.tile([P, KD * P], f32r, tag="w1p")
                nc.sync.dma_start(w1p[:], w1[m])
                h1m = h1pool.tile([P, Nc], f32r, tag=f"h1_{m}")
                for cs, cn in cgs:
                    ps = pspool.tile([P, cn], f32, tag="ps")
                    for k in range(KD):
                        nc.tensor.matmul(
                            ps[:],
                            lhsT=w1p[:, k * P : (k + 1) * P],
                            rhs=x_sb[k][:, cs : cs + cn],
                            start=(k == 0),
                            stop=(k == KD - 1),
                        )
                    nc.scalar.activation(
                        h1m[:, cs : cs + cn],
                        ps[:],
                        GELU,
                        bias=(b1_sb[:, m : m + 1] if has_b1 else 0.0),
                    )
                h1_sb.append(h1m)

            # prefetch next chunk's activations; their x slots free as soon
            # as this chunk's L1 matmuls finish, so the DMA lands during L2
            if ci + 1 < len(chunks):
                pending = load_chunk_inputs(ci + 1)

            # ---- L2: h2 = gelu(h1 @ W2 + b2), feature-major [H, Nc] ----
            h2_sb = []
            for m in range(KH):
                w2p = w2pool.tile([P, KI * P], f32r, tag="w2p")
                nc.sync.dma_start(w2p[:], w2[m])
                h2m = h2pool.tile([P, Nc], f32r, tag=f"h2_{m}")
                for cs, cn in cgs:
                    ps = pspool.tile([P, cn], f32, tag="ps")
                    for k in range(KI):
                        nc.tensor.matmul(
                            ps[:],
                            lhsT=w2p[:, k * P : (k + 1) * P],
                            rhs=h1_sb[k][:, cs : cs + cn],
                            start=(k == 0),
                            stop=(k == KI - 1),
                        )
                    nc.scalar.activation(
                        h2m[:, cs : cs + cn],
                        ps[:],
                        GELU,
                        bias=(b2_sb[:, m : m + 1] if has_b2 else 0.0),
                    )
                h2_sb.append(h2m)

            # ---- L3: y = (h2 @ W3 + b3) * gate, feature-major [D, Nc] ----
            for m in range(KD):
                w3p = w3pool.tile([P, KH * P], f32r, tag="w3p")
                nc.sync.dma_start(w3p[:], w3[m])
                y_sb = ypool.tile([P, Nc], f32, tag="y")
                for cs, cn in cgs:
                    ps = pspool.tile([P, cn], f32, tag="ps")
                    for k in range(KH):
                        nc.tensor.matmul(
                            ps[:],
                            lhsT=w3p[:, k * P : (k + 1) * P],
                            rhs=h2_sb[k][:, cs : cs + cn],
                            start=(k == 0),
                            stop=(k == KH - 1),
                        )
                    if has_b3:
                        nc.vector.tensor_scalar_add(
                            y_sb[:, cs : cs + cn], ps[:], b3_sb[:, m : m + 1]
                        )
                        nc.vector.tensor_mul(
                            y_sb[:, cs : cs + cn],
                            y_sb[:, cs : cs + cn],
                            g_bc[:, cs : cs + cn],
                        )
                    else:
                        nc.vector.tensor_mul(
                            y_sb[:, cs : cs + cn], ps[:], g_bc[:, cs : cs + cn]
                        )
                yoff = (base * D) + m * P * Nc
                nc.scalar.dma_start(
                    yT[yoff : yoff + P * Nc].rearrange("(p f) -> p f", f=Nc), y_sb[:]
                )

    nc.compile()
    return nc


def _route(x, Wr, br, top_k):
    """Host router: fp32 logits, stable top-k, softmax weights."""
    xt = np.ascontiguousarray(x.reshape(T, D), dtype=np.float32)
    logits = (xt @ np.asarray(Wr, np.float32)) + np.asarray(br, np.float32)
    k = int(top_k)
    # descending by value, ties -> lower index (matches jax.lax.top_k)
    order = np.argsort(-logits, axis=1, kind="stable")[:, :k]  # [T, k]
    vals = np.take_along_axis(logits, order, axis=1)
    vmax = vals.max(axis=1, keepdims=True)
    ex = np.exp(vals - vmax)
    wts = (ex / ex.sum(axis=1, keepdims=True)).astype(np.float32)
    return xt, order, wts


def kernel(x, Wr, br, W1, b1, W2, b2, W3, b3, top_k):
    global LAST_RESULTS
    import os

    from concourse import bass_utils

    x = np.asarray(x)
    out_dtype = x.dtype
    xt, sel, wts = _route(x, Wr, br, top_k)

    W1 = np.asarray(W1, np.float32)
    W2 = np.asarray(W2, np.float32)
    W3 = np.asarray(W3, np.float32)
    b1 = np.asarray(b1, np.float32)
    b2 = np.asarray(b2, np.float32)
    b3 = np.asarray(b3, np.float32)

    # token lists per expert
    idx_e = []
    gate_e = []
    for e in range(E):
        rows, cols = np.nonzero(sel == e)
        idx_e.append(rows)
        gate_e.append(wts[rows, cols])
    counts = np.array([len(i) for i in idx_e])

    C = max(DEFAULT_CAP, int(-(-counts.max() // P) * P), 256)

    has_b1 = bool(np.any(b1))
    has_b2 = bool(np.any(b2))
    has_b3 = bool(np.any(b3))

    key = (C, has_b1, has_b2, has_b3)
    if key not in _PROGRAM_CACHE:
        _PROGRAM_CACHE[key] = _build_program(C, has_b1, has_b2, has_b3)
    nc = _PROGRAM_CACHE[key]

    # biggest expert goes to core 0 (the traced core)
    perm = np.argsort(-counts, kind="stable")

    chunks = _chunk_sizes(C)
    bases = []
    b_ = 0
    for s in chunks:
        bases.append(b_)
        b_ += s

    def pack_x(xTe):
        """[D, C] -> flat chunked layout: per chunk, KD blocks of [128, Nc]."""
        parts = [
            np.ascontiguousarray(xTe[:, base : base + Nc]).reshape(-1)
            for base, Nc in zip(bases, chunks)
        ]
        return np.concatenate(parts)

    def tile_w(w, km):
        """[K, M] -> [km_panels, 128, K] panel-contiguous layout."""
        K, M = w.shape
        # panel m: element (p, a*128+f) = w[a*128+p, m*128+f]
        v = w.reshape(K // P, P, km, P)  # [a, p, m, f]
        return np.ascontiguousarray(v.transpose(2, 1, 0, 3)).reshape(km, P, K)

    def unpack_y(flat):
        """flat chunked layout -> [D, C]"""
        out = np.empty((D, C), np.float32)
        for base, Nc in zip(bases, chunks):
            blk = flat[base * D : (base + Nc) * D].reshape(KD, P, Nc)
            out[:, base : base + Nc] = blk.reshape(D, Nc)
        return out

    in_maps = []
    for i in range(N_CORES):
        e = int(perm[i])
        n = int(counts[e])
        xTe = np.zeros((D, C), np.float32)
        xTe[:, :n] = xt[idx_e[e]].T
        g = np.zeros((C,), np.float32)
        g[:n] = gate_e[e]
        m = {
            "xTt": pack_x(xTe),
            "w1t": tile_w(W1[e], KI),
            "w2t": tile_w(W2[e], KH),
            "w3t": tile_w(W3[e], KD),
            "gate": g,
        }
        if has_b1:
            m["b1t"] = np.ascontiguousarray(b1[e].reshape(KI, P).T)
        if has_b2:
            m["b2t"] = np.ascontiguousarray(b2[e].reshape(KH, P).T)
        if has_b3:
            m["b3t"] = np.ascontiguousarray(b3[e].reshape(KD, P).T)
        in_maps.append(m)

    trace_cores = None
    if os.environ.get("BASS_TRACE"):
        trace_cores = [0]

    res = bass_utils.run_bass_kernel_spmd(
        nc,
        in_maps,
        core_ids=list(range(N_CORES)),
        trace_cores=trace_cores,
    )
    LAST_RESULTS = res

    out = np.zeros((T, D), np.float32)
    core_of_expert = {int(perm[i]): i for i in range(N_CORES)}
    for e in range(E):  # expert-ascending to match reference summation order
        i = core_of_expert[e]
        n = int(counts[e])
        if n:
            yTe = unpack_y(res.results[i]["yTt"])
            out[idx_e[e]] += yTe[:, :n].T
    return out.reshape(B, S, D).astype(out_dtype, copy=False)
